# revision 14
# baseline (speedup 1.0000x reference)
"""Causal self-attention (B=2, T=2048, C=2048, NH=16) on 8 TRN2 NeuronCores.

Megatron-style tensor parallelism over heads: each core owns 2 heads.
All matmul operands are bf16 (PE rate equals fp32r at these shapes, but
DMA/SBUF halve); PSUM accumulation stays fp32.

Per core, fully fused single pass over 8 token chunks of 512:
  - QKV projection chunk-by-chunk, q/k/v kept SBUF-resident (no spills).
    Weights are loaded in output-column slices so the first matmul starts
    ~3us after kernel entry.
  - Attention interleaved per 512-query chunk right behind the QKV chunk
    that completes its causal k-prefix: S^T tiles = k_tile.T @ q_chunk,
    exp on ScalarE (PSUM->SBUF, bf16 out), one 0/1 mask multiply per
    (head, chunk) on the 4 diagonal k-tiles, softmax denominator via
    k-tile pair-sums on the (otherwise idle) Pool engine followed by a
    half-length all-ones matmul, O^T = V-stationary accumulation, divide
    by denominator on DVE.
  - Output projection per chunk from SBUF-resident y (contract the 256
    local head channels against w_proj columns), partial [512, 2048]
    written to DRAM, then a per-chunk ReduceScatter(add) across the 8
    cores produces each core's final 64-token slice. The last chunk
    reduce-scatters per 128-token tile to shrink the drain tail.
Denominator/AV/out-proj matmuls are emitted as small FIFO fragments
interleaved between later S-matmul groups so the in-order PE queue never
head-of-line blocks on the exp pipeline. All PSUM tiles used by deferred
fragments are allocated inside the fragment (emission order == pool
rotation order).
Host side: cast/shard inputs to bf16, reassemble the scattered output.
"""

import numpy as np
import ml_dtypes

import concourse.bacc as bacc
import concourse.mybir as mybir
import concourse.tile as tile
from concourse.bass_utils import run_bass_kernel_spmd
from concourse.hw_specs import get_activation_tables as _get_act_tables


def _act_tables_pin_exp_ln(arch):
    """Resolve Exp and Ln only via the combined natural_log_exp set so the
    kernel never pays an ACT table reload when alternating exp/ln."""
    t = _get_act_tables(arch)
    for name, fns in t.items():
        if name != "natural_log_exp_and_others":
            fns.discard(mybir.ActivationFunctionType.Exp)
            fns.discard(mybir.ActivationFunctionType.Ln)
    return t


bacc.get_activation_tables = _act_tables_pin_exp_ln

BF16 = mybir.dt.bfloat16
F32 = mybir.dt.float32
EXP = mybir.ActivationFunctionType.Exp
LN = mybir.ActivationFunctionType.Ln

B, T, C, NH, HS = 2, 2048, 2048, 16, 128
NCORES = 8
HPC = NH // NCORES          # heads per core
BT = B * T                  # 4096 tokens total
CT = C // 128               # 16 contraction tiles
TCH = 512                   # token chunk (both projection and query chunk)
NCH = BT // TCH             # 8 chunks
NQC = T // TCH              # 4 query chunks per batch
NOS = 4                     # out-proj output-column slices
OSS = C // NOS              # 512
TPC = TCH // NCORES         # 64 final tokens per core per chunk


def build_nc(cc: bool = True):
    nc = bacc.Bacc("TRN2", target_bir_lowering=False, num_devices=NCORES)

    # host-blocked so every load is 128 fat contiguous descriptors
    xT = nc.dram_tensor("xT", [NCH, 128, CT, TCH], BF16, kind="ExternalInput")
    # q/k weights in output-column slices; v weights separate
    wqT = nc.dram_tensor("wqT", [128, 4, CT, 128], BF16, kind="ExternalInput")
    wvT = nc.dram_tensor("wvT", [128, CT, HPC * HS], BF16, kind="ExternalInput")
    wpT = nc.dram_tensor("wpT", [128, HPC, C], BF16, kind="ExternalInput")
    masks = nc.dram_tensor("masks", [128, 4, TCH], BF16, kind="ExternalInput")
    ones = nc.dram_tensor("ones", [128, 128], BF16, kind="ExternalInput")
    # per-chunk partial output (full 2048 channels) and its reduce-scatter
    pout = [nc.dram_tensor(f"pout{ch}", [TCH, C], BF16) for ch in range(NCH)]
    rs_buf = [
        nc.dram_tensor(f"rs_buf{ch}", [TCH * C // NCORES], BF16)
        for ch in range(NCH - 1)
    ]
    # last chunk reduce-scatters per 128-token tile
    rs_last = [
        nc.dram_tensor(f"rs_last{tt}", [128 * C // NCORES], BF16)
        for tt in range(TCH // 128)
    ]
    rs_out = nc.dram_tensor(
        "rs_out", [NCH, TCH * C // NCORES], BF16, kind="ExternalOutput"
    )

    with tile.TileContext(nc) as tc:
        with (
            tc.tile_pool(name="const", bufs=1) as const,
            tc.tile_pool(name="wqc", bufs=4) as wqc_pool,
            tc.tile_pool(name="wv", bufs=1) as wv_pool,
            tc.tile_pool(name="wp", bufs=1) as wp_pool,
            tc.tile_pool(name="xin", bufs=3) as xin,
            tc.tile_pool(name="qp", bufs=2) as qp,
            tc.tile_pool(name="kp", bufs=2) as kp,
            tc.tile_pool(name="vp", bufs=2) as vp,
            tc.tile_pool(name="esp", bufs=2) as esp,
            tc.tile_pool(name="es2p", bufs=2) as es2p,
            tc.tile_pool(name="yp", bufs=2) as yp,
            tc.tile_pool(name="rp", bufs=2) as rp,
            tc.tile_pool(name="op", bufs=3) as op_pool,
            tc.tile_pool(name="ps_s", bufs=4, space="PSUM") as ps_s,
            tc.tile_pool(name="ps_dp", bufs=1, space="PSUM") as ps_dp,
            tc.tile_pool(name="ps_po", bufs=1, space="PSUM") as ps_po,
            tc.tile_pool(name="ps_pb", bufs=2, space="PSUM") as ps_pb,
        ):
            # startup: x chunk 0 on the sync queue (one fat DMA), weight
            # column-slices on the gpsimd queue — first matmul needs only
            # wq slice 0 + x0, ~1.5MB total across two queues.
            x_tiles: dict = {}
            x_first = xin.tile([128, CT, TCH], BF16, name="x_sb")
            x_tiles[0] = x_first
            nc.sync.dma_start(out=x_first, in_=xT[0])
            wq_c = []
            for ot in range(4):
                w_t = wqc_pool.tile([128, CT, 128], BF16, name="wqc")
                wq_c.append(w_t)
                nc.gpsimd.dma_start(out=w_t, in_=wqT[:, ot])
            wv_sb = wv_pool.tile([128, CT, HPC * HS], BF16)
            nc.gpsimd.dma_start(out=wv_sb, in_=wvT[:])
            masks_sb = const.tile([128, 4, TCH], BF16)
            nc.gpsimd.dma_start(out=masks_sb, in_=masks[:])
            ones_sb = const.tile([128, 128], BF16)
            nc.gpsimd.dma_start(out=ones_sb, in_=ones[:])
            wp_sb = wp_pool.tile([128, HPC, C], BF16)
            nc.gpsimd.dma_start(out=wp_sb, in_=wpT[:])

            # qkv SBUF residency: one tile per batch, rotating bufs=2
            q_sb: dict = {}
            k_sb: dict = {}
            v_sb: dict = {}

            # deferred small PE fragments (denominator / AV / out-proj)
            # popped FIFO between S-matmuls so the PE never runs dry
            pending: list = []

            def pop_pending(n):
                for _ in range(min(n, len(pending))):
                    pending.pop(0)()

            def flush_pending():
                while pending:
                    pending.pop(0)()

            def load_x(tch):
                x_t = xin.tile([128, CT, TCH], BF16, name="x_sb")
                x_tiles[tch] = x_t
                nc.sync.dma_start(out=x_t, in_=xT[tch])

            def qkv_chunk(tch):
                bb, tin = tch // NQC, (tch % NQC) * TCH
                tsl = slice(tin, tin + TCH)
                if bb not in q_sb:
                    q_sb[bb] = qp.tile([128, HPC, T], BF16, name="q_sb")
                    k_sb[bb] = kp.tile([128, HPC, T], BF16, name="k_sb")
                    v_sb[bb] = vp.tile([128, CT, HPC * HS], BF16, name="v_sb")
                x_t = x_tiles.pop(tch)
                for ot in range(4):  # q_h0, q_h1, k_h0, k_h1
                    pq = ps_s.tile([128, TCH], F32, name="sp")
                    for ci in range(CT):
                        nc.tensor.matmul(
                            pq[:],
                            wq_c[ot][:, ci, :],
                            x_t[:, ci, :],
                            start=(ci == 0),
                            stop=(ci == CT - 1),
                        )
                    dst = (q_sb if ot < 2 else k_sb)[bb]
                    nc.vector.tensor_copy(out=dst[:, ot % 2, tsl], in_=pq[:])
                    pop_pending(2)
                for tt in range(TCH // 128):  # V in [token, d] layout
                    pv = ps_pb.tile([128, TCH], F32, name="pb")
                    for ci in range(CT):
                        nc.tensor.matmul(
                            pv[:, : HPC * HS],
                            x_t[:, ci, tt * 128 : (tt + 1) * 128],
                            wv_sb[:, ci, :],
                            start=(ci == 0),
                            stop=(ci == CT - 1),
                        )
                    ktg = (tch % NQC) * 4 + tt
                    nc.vector.tensor_copy(
                        out=v_sb[bb][:, ktg, :], in_=pv[:, : HPC * HS]
                    )
                    pop_pending(2)

            def denom_av(b, hl, nk, es, y_t):
                """Queue pair-sum + denominator + AV + divide for one
                (chunk, head) as small PE fragments. PSUM tiles allocated at
                pop time so pool rotation follows emission order."""
                nk2 = nk // 2
                dp_box: list = []
                po_box: list = []
                r_box: list = []
                es2_box: list = []

                def pair_sum():
                    # halve the denominator matmul by summing k-tile pairs
                    # on the Pool engine (idle otherwise)
                    es2 = es2p.tile([128, CT // 2, TCH], BF16, name="es2")
                    es2_box.append(es2)
                    nc.gpsimd.tensor_tensor(
                        es2[:, :nk2, :],
                        es[:, :nk2, :],
                        es[:, nk2:nk, :],
                        mybir.AluOpType.add,
                    )

                def dp_frag(k0, k1):
                    if not dp_box:
                        dp_box.append(ps_dp.tile([128, TCH], F32, name="dp"))
                    dp = dp_box[0]
                    for kt in range(k0, k1):
                        nc.tensor.matmul(
                            dp[:], ones_sb[:], es2_box[0][:, kt, :],
                            start=(kt == 0), stop=(kt == nk2 - 1),
                            skip_group_check=True,
                        )

                def recip():
                    # 1/x as exp(-ln(x)) on ScalarE (DVE reciprocal is slow)
                    ln_t = rp.tile([128, TCH], F32, tag="lnt", name="ln_sb")
                    nc.scalar.activation(out=ln_t[:], in_=dp_box[0][:], func=LN)
                    r_t = rp.tile([128, TCH], F32, tag="rsb", name="r_sb")
                    nc.scalar.activation(out=r_t[:], in_=ln_t[:], func=EXP, scale=-1.0)
                    r_box.append(r_t)

                def po_frag(k0, k1):
                    if not po_box:
                        po_box.append(ps_po.tile([128, TCH], F32, name="po"))
                    po = po_box[0]
                    for kt in range(k0, k1):
                        nc.tensor.matmul(
                            po[:], v_sb[b][:, kt, hl * HS : (hl + 1) * HS],
                            es[:, kt, :],
                            start=(kt == 0), stop=(kt == nk - 1),
                            skip_group_check=True,
                        )

                def div():
                    nc.vector.tensor_mul(
                        out=y_t[:, hl, :], in0=po_box[0][:], in1=r_box[0][:]
                    )

                pending.append(pair_sum)
                for k0 in range(0, nk2, 4):
                    pending.append(lambda k0=k0: dp_frag(k0, min(k0 + 4, nk2)))
                pending.append(recip)
                for k0 in range(0, nk, 4):
                    pending.append(lambda k0=k0: po_frag(k0, min(k0 + 4, nk)))
                pending.append(div)

            def out_proj(ch, y_t):
                """Queue the chunk's out-projection as per-(tt,os) fragments."""
                last = ch == NCH - 1
                o_tiles: dict = {}

                def frag(tt, osl):
                    if osl == 0:
                        o_tiles[tt] = op_pool.tile([128, C], BF16, name="o_sb")
                    po3 = ps_pb.tile([128, TCH], F32, name="pb")
                    for hl in range(HPC):
                        nc.tensor.matmul(
                            po3[:],
                            y_t[:, hl, tt * 128 : (tt + 1) * 128],
                            wp_sb[:, hl, osl * OSS : (osl + 1) * OSS],
                            start=(hl == 0),
                            stop=(hl == HPC - 1),
                        )
                    dst = o_tiles[tt][:, osl * OSS : (osl + 1) * OSS]
                    if osl < 2:
                        nc.vector.tensor_copy(out=dst, in_=po3[:])
                    else:
                        nc.scalar.copy(out=dst, in_=po3[:])
                    if osl == NOS - 1:
                        nc.sync.dma_start(
                            out=pout[ch][tt * 128 : (tt + 1) * 128, :],
                            in_=o_tiles[tt],
                        )
                        if last:
                            rs_tt(tt)

                def rs_tt(tt):
                    if cc:
                        nc.gpsimd.collective_compute(
                            "ReduceScatter",
                            mybir.AluOpType.add,
                            replica_groups=[list(range(NCORES))],
                            ins=[pout[ch][tt * 128 : (tt + 1) * 128, :]],
                            outs=[rs_last[tt].ap()],
                        )
                        nc.gpsimd.dma_start(
                            out=rs_out[
                                ch,
                                tt * 128 * C // NCORES : (tt + 1)
                                * 128
                                * C
                                // NCORES,
                            ],
                            in_=rs_last[tt].ap(),
                        )
                    else:
                        nc.sync.dma_start(
                            out=rs_out[
                                ch,
                                tt * 128 * C // NCORES : (tt + 1)
                                * 128
                                * C
                                // NCORES,
                            ].rearrange("(a b) -> a b", b=C),
                            in_=pout[ch][tt * 128 : tt * 128 + 128 // NCORES, :],
                        )

                def rs():
                    if cc:
                        nc.gpsimd.collective_compute(
                            "ReduceScatter",
                            mybir.AluOpType.add,
                            replica_groups=[list(range(NCORES))],
                            ins=[pout[ch].ap()],
                            outs=[rs_buf[ch].ap()],
                        )
                        nc.gpsimd.dma_start(
                            out=rs_out[ch], in_=rs_buf[ch].ap()
                        )
                    else:  # timing-only variant: no inter-core traffic
                        nc.sync.dma_start(
                            out=rs_out[ch].rearrange("(a b) -> a b", b=C),
                            in_=pout[ch][:TPC, :],
                        )

                for tt in range(TCH // 128):
                    for osl in range(NOS):
                        pending.append(lambda tt=tt, osl=osl: frag(tt, osl))
                if not last:
                    pending.append(rs)

            def attn_chunk(b, qc):
                nk = (qc + 1) * (TCH // 128)  # causal: k-tiles 0..nk-1
                qsl = slice(qc * TCH, (qc + 1) * TCH)
                y_t = yp.tile([128, HPC, TCH], BF16, name="y_sb")
                for hl in range(HPC):
                    es = esp.tile([128, CT, TCH], BF16, name="es")
                    for kt in range(nk):
                        sp = ps_s.tile([128, TCH], F32, name="sp")
                        nc.tensor.matmul(
                            sp[:],
                            k_sb[b][:, hl, kt * 128 : (kt + 1) * 128],
                            q_sb[b][:, hl, qsl],
                            start=True,
                            stop=True,
                        )
                        nc.scalar.activation(out=es[:, kt, :], in_=sp[:], func=EXP)
                        pop_pending(2)
                    # one 0/1 mask multiply over the 4 diagonal k-tiles
                    nc.vector.tensor_tensor(
                        es[:, nk - 4 : nk, :],
                        es[:, nk - 4 : nk, :],
                        masks_sb[:],
                        mybir.AluOpType.mult,
                    )
                    denom_av(b, hl, nk, es, y_t)
                out_proj(b * NQC + qc, y_t)

            # ---------------- schedule ----------------
            load_x(1)
            for tch in range(NCH):
                if tch + 2 < NCH:
                    load_x(tch + 2)
                qkv_chunk(tch)
                attn_chunk(tch // NQC, tch % NQC)
            flush_pending()

    nc.finalize()
    return nc


def prep_inputs(x: np.ndarray, w_attn: np.ndarray, w_proj: np.ndarray):
    """Host-side sharding/layout. Returns per-core input maps."""
    bf = ml_dtypes.bfloat16
    xT = np.ascontiguousarray(
        x.reshape(NCH, TCH, CT, 128).transpose(0, 3, 2, 1)
    ).astype(bf)
    wq, wk, wv = w_attn[:C], w_attn[C : 2 * C], w_attn[2 * C :]
    scale = np.float32(1.0 / np.sqrt(HS))
    kk = np.arange(128, dtype=np.int64)[:, None, None]
    aa = np.arange(4, dtype=np.int64)[None, :, None]
    qq = np.arange(TCH, dtype=np.int64)[None, None, :]
    masks = (128 * aa + kk <= qq).astype(bf)
    ones = np.ones((128, 128), dtype=bf)
    in_maps = []
    for c in range(NCORES):
        h0 = HPC * c
        rows = slice(h0 * HS, (h0 + HPC) * HS)
        wqk = np.concatenate([wq[rows] * scale, wk[rows]], axis=0)  # [512, C]
        # [128p, 4 slices, CT, 128 outcols]
        wqT = np.ascontiguousarray(
            wqk.T.reshape(CT, 128, 4, 128).transpose(1, 2, 0, 3)
        ).astype(bf)
        wvT = np.ascontiguousarray(
            wv[rows].T.reshape(CT, 128, HPC * HS).transpose(1, 0, 2)
        ).astype(bf)
        # wpT[c]: rows = this core's 256 y channels, all 2048 out channels
        wpT = np.ascontiguousarray(
            w_proj[:, c * HPC * HS : (c + 1) * HPC * HS]
            .T.reshape(HPC, 128, C)
            .transpose(1, 0, 2)
        ).astype(bf)
        in_maps.append(
            {
                "xT": xT,
                "wqT": wqT,
                "wvT": wvT,
                "wpT": wpT,
                "masks": masks,
                "ones": ones,
            }
        )
    return in_maps


_CACHE: dict = {}


def _get_nc(cc: bool = True):
    key = ("nc", cc)
    if key not in _CACHE:
        _CACHE[key] = build_nc(cc=cc)
    return _CACHE[key]


def run(x, w_attn, w_proj, cc: bool = True, **spmd_kwargs):
    nc = _get_nc(cc=cc)
    in_maps = prep_inputs(
        np.asarray(x, dtype=np.float32),
        np.asarray(w_attn, dtype=np.float32),
        np.asarray(w_proj, dtype=np.float32),
    )
    res = run_bass_kernel_spmd(nc, in_maps, list(range(NCORES)), **spmd_kwargs)
    # rs_out[c][ch] holds tokens [64c .. 64c+64) of chunk ch (for the last
    # chunk: tokens [16c .. 16c+16) of each 128-token tile)
    out = np.zeros((BT, C), dtype=np.float32)
    for c in range(NCORES):
        r = np.asarray(res.results[c]["rs_out"], dtype=np.float32)
        for ch in range(NCH - 1):
            t0 = ch * TCH + c * TPC
            out[t0 : t0 + TPC, :] = r[ch].reshape(TPC, C)
        ch = NCH - 1
        rl = r[ch].reshape(4, 128 // NCORES, C)
        for tt in range(4):
            t0 = ch * TCH + tt * 128 + c * (128 // NCORES)
            out[t0 : t0 + 128 // NCORES, :] = rl[tt]
    return out.reshape(B, T, C), res


def kernel(x, w_attn, w_proj):
    out, _ = run(x, w_attn, w_proj, cc=True)
    return out


# revision 20
# speedup vs baseline: 1.1532x; 1.1532x over previous
"""Causal self-attention (B=2, T=2048, C=2048, NH=16) on 8 TRN2 NeuronCores.

Megatron-style tensor parallelism over heads: each core owns 2 heads.
All matmul operands are bf16 (PE rate equals fp32r at these shapes, but
DMA/SBUF halve); PSUM accumulation stays fp32.

Per core, fully fused single pass over 8 token chunks of 512:
  - QKV projection chunk-by-chunk, q/k/v kept SBUF-resident (no spills).
    Weights are loaded in output-column slices so the first matmul starts
    ~3us after kernel entry.
  - Attention interleaved per 512-query chunk right behind the QKV chunk
    that completes its causal k-prefix: S^T tiles = k_tile.T @ q_chunk,
    exp on ScalarE (PSUM->SBUF, bf16 out), one 0/1 mask multiply per
    (head, chunk) on the 4 diagonal k-tiles, softmax denominator via
    k-tile pair-sums on the (otherwise idle) Pool engine followed by a
    half-length all-ones matmul, O^T = V-stationary accumulation, divide
    by denominator on DVE.
  - Output projection per chunk from SBUF-resident y (contract the 256
    local head channels against w_proj columns), partial [512, 2048]
    written to DRAM, then a per-chunk ReduceScatter(add) across the 8
    cores produces each core's final 64-token slice. The last chunk
    reduce-scatters per 128-token tile to shrink the drain tail.
Denominator/AV/out-proj matmuls are emitted as small FIFO fragments
interleaved between later S-matmul groups so the in-order PE queue never
head-of-line blocks on the exp pipeline. All PSUM tiles used by deferred
fragments are allocated inside the fragment (emission order == pool
rotation order).
Host side: cast/shard inputs to bf16, reassemble the scattered output.
"""

import numpy as np
import ml_dtypes

import concourse.bacc as bacc
import concourse.mybir as mybir
import concourse.tile as tile
from concourse.bass_utils import run_bass_kernel_spmd
from concourse.hw_specs import get_activation_tables as _get_act_tables


def _act_tables_pin_exp_ln(arch):
    """Resolve Exp and Ln only via the combined natural_log_exp set so the
    kernel never pays an ACT table reload when alternating exp/ln."""
    t = _get_act_tables(arch)
    for name, fns in t.items():
        if name != "natural_log_exp_and_others":
            fns.discard(mybir.ActivationFunctionType.Exp)
            fns.discard(mybir.ActivationFunctionType.Ln)
    return t


bacc.get_activation_tables = _act_tables_pin_exp_ln

BF16 = mybir.dt.bfloat16
F32 = mybir.dt.float32
EXP = mybir.ActivationFunctionType.Exp
LN = mybir.ActivationFunctionType.Ln

B, T, C, NH, HS = 2, 2048, 2048, 16, 128
NCORES = 8
HPC = NH // NCORES          # heads per core
BT = B * T                  # 4096 tokens total
CT = C // 128               # 16 contraction tiles
TCH = 512                   # token chunk (both projection and query chunk)
NCH = BT // TCH             # 8 chunks
NQC = T // TCH              # 4 query chunks per batch
NOS = 4                     # out-proj output-column slices
OSS = C // NOS              # 512
TPC = TCH // NCORES         # 64 final tokens per core per chunk


def build_nc(cc: bool = True):
    nc = bacc.Bacc("TRN2", target_bir_lowering=False, num_devices=NCORES)

    # host-blocked so every load is 128 fat contiguous descriptors
    xT = nc.dram_tensor("xT", [NCH, 128, CT, TCH], BF16, kind="ExternalInput")
    # q/k weights in output-column slices; v weights separate
    wqT = nc.dram_tensor("wqT", [128, 4, CT, 128], BF16, kind="ExternalInput")
    wvT = nc.dram_tensor("wvT", [128, CT, HPC * HS], BF16, kind="ExternalInput")
    wpT = nc.dram_tensor("wpT", [128, HPC, C], BF16, kind="ExternalInput")
    masks = nc.dram_tensor("masks", [128, 4, TCH], BF16, kind="ExternalInput")
    ones = nc.dram_tensor("ones", [128, 128], BF16, kind="ExternalInput")
    # per-chunk partial output (full 2048 channels) and its reduce-scatter
    pout = [nc.dram_tensor(f"pout{ch}", [TCH, C], BF16) for ch in range(NCH)]
    rs_buf = [
        nc.dram_tensor(f"rs_buf{ch}", [TCH * C // NCORES], BF16)
        for ch in range(NCH - 1)
    ]
    # last chunk reduce-scatters per 128-token tile
    rs_last = [
        nc.dram_tensor(f"rs_last{tt}", [128 * C // NCORES], BF16)
        for tt in range(TCH // 128)
    ]
    rs_out = nc.dram_tensor(
        "rs_out", [NCH, TCH * C // NCORES], BF16, kind="ExternalOutput"
    )

    with tile.TileContext(nc) as tc:
        with (
            tc.tile_pool(name="const", bufs=1) as const,
            tc.tile_pool(name="wqc", bufs=4) as wqc_pool,
            tc.tile_pool(name="wv", bufs=1) as wv_pool,
            tc.tile_pool(name="wp", bufs=1) as wp_pool,
            tc.tile_pool(name="xin", bufs=3) as xin,
            tc.tile_pool(name="qp", bufs=2) as qp,
            tc.tile_pool(name="kp", bufs=2) as kp,
            tc.tile_pool(name="vp", bufs=2) as vp,
            tc.tile_pool(name="esp", bufs=2) as esp,
            tc.tile_pool(name="es2p", bufs=2) as es2p,
            tc.tile_pool(name="yp", bufs=2) as yp,
            tc.tile_pool(name="rp", bufs=2) as rp,
            tc.tile_pool(name="op", bufs=3) as op_pool,
            tc.tile_pool(name="ps_s", bufs=4, space="PSUM") as ps_s,
            tc.tile_pool(name="ps_dp", bufs=1, space="PSUM") as ps_dp,
            tc.tile_pool(name="ps_po", bufs=1, space="PSUM") as ps_po,
            tc.tile_pool(name="ps_pb", bufs=2, space="PSUM") as ps_pb,
        ):
            # startup: everything on the sync queue (hardware DGE — the
            # gpsimd software DGE runs at ~1/10 the bandwidth), ordered by
            # first use: wq slice 0 + x0 gate the first matmul.
            x_tiles: dict = {}
            wq_c = [
                wqc_pool.tile([128, CT, 128], BF16, name="wqc")
                for ot in range(4)
            ]
            nc.sync.dma_start(out=wq_c[0], in_=wqT[:, 0])
            x_first = xin.tile([128, CT, TCH], BF16, name="x_sb")
            x_tiles[0] = x_first
            nc.sync.dma_start(out=x_first, in_=xT[0])
            for ot in range(1, 4):
                nc.sync.dma_start(out=wq_c[ot], in_=wqT[:, ot])
            masks_sb = const.tile([128, 4, TCH], BF16)
            nc.sync.dma_start(out=masks_sb, in_=masks[:])
            wv_sb = wv_pool.tile([128, CT, HPC * HS], BF16)
            nc.sync.dma_start(out=wv_sb, in_=wvT[:])
            ones_sb = const.tile([128, 128], BF16)
            nc.sync.dma_start(out=ones_sb, in_=ones[:])
            wp_sb = wp_pool.tile([128, HPC, C], BF16)

            # qkv SBUF residency: one tile per batch, rotating bufs=2
            q_sb: dict = {}
            k_sb: dict = {}
            v_sb: dict = {}

            # deferred small PE fragments (denominator / AV / out-proj)
            # popped FIFO between S-matmuls so the PE never runs dry
            pending: list = []

            def pop_pending(n):
                for _ in range(min(n, len(pending))):
                    pending.pop(0)()

            def flush_pending():
                while pending:
                    pending.pop(0)()

            def load_x(tch):
                x_t = xin.tile([128, CT, TCH], BF16, name="x_sb")
                x_tiles[tch] = x_t
                nc.sync.dma_start(out=x_t, in_=xT[tch])

            def qkv_chunk(tch):
                bb, tin = tch // NQC, (tch % NQC) * TCH
                tsl = slice(tin, tin + TCH)
                if bb not in q_sb:
                    q_sb[bb] = qp.tile([128, HPC, T], BF16, name="q_sb")
                    k_sb[bb] = kp.tile([128, HPC, T], BF16, name="k_sb")
                    v_sb[bb] = vp.tile([128, CT, HPC * HS], BF16, name="v_sb")
                x_t = x_tiles.pop(tch)
                for ot in range(4):  # q_h0, q_h1, k_h0, k_h1
                    pq = ps_s.tile([128, TCH], F32, name="sp")
                    for ci in range(CT):
                        nc.tensor.matmul(
                            pq[:],
                            wq_c[ot][:, ci, :],
                            x_t[:, ci, :],
                            start=(ci == 0),
                            stop=(ci == CT - 1),
                        )
                    dst = (q_sb if ot < 2 else k_sb)[bb]
                    nc.vector.tensor_copy(out=dst[:, ot % 2, tsl], in_=pq[:])
                    pop_pending(2)
                for tt in range(TCH // 128):  # V in [token, d] layout
                    pv = ps_pb.tile([128, TCH], F32, name="pb")
                    for ci in range(CT):
                        nc.tensor.matmul(
                            pv[:, : HPC * HS],
                            x_t[:, ci, tt * 128 : (tt + 1) * 128],
                            wv_sb[:, ci, :],
                            start=(ci == 0),
                            stop=(ci == CT - 1),
                        )
                    ktg = (tch % NQC) * 4 + tt
                    nc.vector.tensor_copy(
                        out=v_sb[bb][:, ktg, :], in_=pv[:, : HPC * HS]
                    )
                    pop_pending(2)

            def denom_av(b, hl, nk, es, y_t):
                """Queue pair-sum + denominator + AV + divide for one
                (chunk, head) as small PE fragments. PSUM tiles allocated at
                pop time so pool rotation follows emission order."""
                nk2 = nk // 2
                dp_box: list = []
                po_box: list = []
                r_box: list = []
                # halve the denominator matmul by summing k-tile pairs on
                # DVE, emitted inline (before the dp fragments pop) so the
                # dp matmuls never head-of-line block the PE queue
                es2 = es2p.tile([128, CT // 2, TCH], BF16, name="es2")
                nc.vector.tensor_tensor(
                    es2[:, :nk2, :],
                    es[:, :nk2, :],
                    es[:, nk2:nk, :],
                    mybir.AluOpType.add,
                )

                def dp_frag(k0, k1):
                    if not dp_box:
                        dp_box.append(ps_dp.tile([128, TCH], F32, name="dp"))
                    dp = dp_box[0]
                    for kt in range(k0, k1):
                        nc.tensor.matmul(
                            dp[:], ones_sb[:], es2[:, kt, :],
                            start=(kt == 0), stop=(kt == nk2 - 1),
                            skip_group_check=True,
                        )

                def recip():
                    # 1/x as exp(-ln(x)) on ScalarE (DVE reciprocal is slow)
                    ln_t = rp.tile([128, TCH], F32, tag="lnt", name="ln_sb")
                    nc.scalar.activation(out=ln_t[:], in_=dp_box[0][:], func=LN)
                    r_t = rp.tile([128, TCH], F32, tag="rsb", name="r_sb")
                    nc.scalar.activation(out=r_t[:], in_=ln_t[:], func=EXP, scale=-1.0)
                    r_box.append(r_t)

                def po_frag(k0, k1):
                    if not po_box:
                        po_box.append(ps_po.tile([128, TCH], F32, name="po"))
                    po = po_box[0]
                    for kt in range(k0, k1):
                        nc.tensor.matmul(
                            po[:], v_sb[b][:, kt, hl * HS : (hl + 1) * HS],
                            es[:, kt, :],
                            start=(kt == 0), stop=(kt == nk - 1),
                            skip_group_check=True,
                        )

                def div():
                    nc.vector.tensor_mul(
                        out=y_t[:, hl, :], in0=po_box[0][:], in1=r_box[0][:]
                    )

                for k0 in range(0, nk2, 4):
                    pending.append(lambda k0=k0: dp_frag(k0, min(k0 + 4, nk2)))
                pending.append(recip)
                for k0 in range(0, nk, 4):
                    pending.append(lambda k0=k0: po_frag(k0, min(k0 + 4, nk)))
                pending.append(div)

            def out_proj(ch, y_t):
                """Queue the chunk's out-projection as per-(tt,os) fragments."""
                last = ch == NCH - 1
                o_tiles: dict = {}

                def frag(tt, osl):
                    if osl == 0:
                        o_tiles[tt] = op_pool.tile([128, C], BF16, name="o_sb")
                    po3 = ps_pb.tile([128, TCH], F32, name="pb")
                    for hl in range(HPC):
                        nc.tensor.matmul(
                            po3[:],
                            y_t[:, hl, tt * 128 : (tt + 1) * 128],
                            wp_sb[:, hl, osl * OSS : (osl + 1) * OSS],
                            start=(hl == 0),
                            stop=(hl == HPC - 1),
                        )
                    dst = o_tiles[tt][:, osl * OSS : (osl + 1) * OSS]
                    if osl < 2:
                        nc.vector.tensor_copy(out=dst, in_=po3[:])
                    else:
                        nc.scalar.copy(out=dst, in_=po3[:])
                    if osl == NOS - 1:
                        nc.sync.dma_start(
                            out=pout[ch][tt * 128 : (tt + 1) * 128, :],
                            in_=o_tiles[tt],
                        )
                        if last:
                            rs_tt(tt)

                def rs_tt(tt):
                    if cc:
                        nc.gpsimd.collective_compute(
                            "ReduceScatter",
                            mybir.AluOpType.add,
                            replica_groups=[list(range(NCORES))],
                            ins=[pout[ch][tt * 128 : (tt + 1) * 128, :]],
                            outs=[rs_last[tt].ap()],
                        )
                        nc.gpsimd.dma_start(
                            out=rs_out[
                                ch,
                                tt * 128 * C // NCORES : (tt + 1)
                                * 128
                                * C
                                // NCORES,
                            ],
                            in_=rs_last[tt].ap(),
                        )
                    else:
                        nc.sync.dma_start(
                            out=rs_out[
                                ch,
                                tt * 128 * C // NCORES : (tt + 1)
                                * 128
                                * C
                                // NCORES,
                            ].rearrange("(a b) -> a b", b=C),
                            in_=pout[ch][tt * 128 : tt * 128 + 128 // NCORES, :],
                        )

                def rs():
                    if cc:
                        nc.gpsimd.collective_compute(
                            "ReduceScatter",
                            mybir.AluOpType.add,
                            replica_groups=[list(range(NCORES))],
                            ins=[pout[ch].ap()],
                            outs=[rs_buf[ch].ap()],
                        )
                        nc.gpsimd.dma_start(
                            out=rs_out[ch], in_=rs_buf[ch].ap()
                        )
                    else:  # timing-only variant: no inter-core traffic
                        nc.sync.dma_start(
                            out=rs_out[ch].rearrange("(a b) -> a b", b=C),
                            in_=pout[ch][:TPC, :],
                        )

                for tt in range(TCH // 128):
                    for osl in range(NOS):
                        pending.append(lambda tt=tt, osl=osl: frag(tt, osl))
                if not last:
                    pending.append(rs)

            def attn_chunk(b, qc):
                nk = (qc + 1) * (TCH // 128)  # causal: k-tiles 0..nk-1
                qsl = slice(qc * TCH, (qc + 1) * TCH)
                y_t = yp.tile([128, HPC, TCH], BF16, name="y_sb")
                for hl in range(HPC):
                    es = esp.tile([128, CT, TCH], BF16, name="es")
                    for kt in range(nk):
                        sp = ps_s.tile([128, TCH], F32, name="sp")
                        nc.tensor.matmul(
                            sp[:],
                            k_sb[b][:, hl, kt * 128 : (kt + 1) * 128],
                            q_sb[b][:, hl, qsl],
                            start=True,
                            stop=True,
                        )
                        nc.scalar.activation(out=es[:, kt, :], in_=sp[:], func=EXP)
                        pop_pending(2)
                    # one 0/1 mask multiply over the 4 diagonal k-tiles
                    nc.vector.tensor_tensor(
                        es[:, nk - 4 : nk, :],
                        es[:, nk - 4 : nk, :],
                        masks_sb[:],
                        mybir.AluOpType.mult,
                    )
                    denom_av(b, hl, nk, es, y_t)
                out_proj(b * NQC + qc, y_t)

            # ---------------- schedule ----------------
            load_x(1)
            nc.sync.dma_start(out=wp_sb, in_=wpT[:])
            for tch in range(NCH):
                if tch + 2 < NCH:
                    load_x(tch + 2)
                qkv_chunk(tch)
                attn_chunk(tch // NQC, tch % NQC)
            flush_pending()

    nc.finalize()
    return nc


def prep_inputs(x: np.ndarray, w_attn: np.ndarray, w_proj: np.ndarray):
    """Host-side sharding/layout. Returns per-core input maps."""
    bf = ml_dtypes.bfloat16
    xT = np.ascontiguousarray(
        x.reshape(NCH, TCH, CT, 128).transpose(0, 3, 2, 1)
    ).astype(bf)
    wq, wk, wv = w_attn[:C], w_attn[C : 2 * C], w_attn[2 * C :]
    scale = np.float32(1.0 / np.sqrt(HS))
    kk = np.arange(128, dtype=np.int64)[:, None, None]
    aa = np.arange(4, dtype=np.int64)[None, :, None]
    qq = np.arange(TCH, dtype=np.int64)[None, None, :]
    masks = (128 * aa + kk <= qq).astype(bf)
    ones = np.ones((128, 128), dtype=bf)
    in_maps = []
    for c in range(NCORES):
        h0 = HPC * c
        rows = slice(h0 * HS, (h0 + HPC) * HS)
        wqk = np.concatenate([wq[rows] * scale, wk[rows]], axis=0)  # [512, C]
        # [128p, 4 slices, CT, 128 outcols]
        wqT = np.ascontiguousarray(
            wqk.T.reshape(CT, 128, 4, 128).transpose(1, 2, 0, 3)
        ).astype(bf)
        wvT = np.ascontiguousarray(
            wv[rows].T.reshape(CT, 128, HPC * HS).transpose(1, 0, 2)
        ).astype(bf)
        # wpT[c]: rows = this core's 256 y channels, all 2048 out channels
        wpT = np.ascontiguousarray(
            w_proj[:, c * HPC * HS : (c + 1) * HPC * HS]
            .T.reshape(HPC, 128, C)
            .transpose(1, 0, 2)
        ).astype(bf)
        in_maps.append(
            {
                "xT": xT,
                "wqT": wqT,
                "wvT": wvT,
                "wpT": wpT,
                "masks": masks,
                "ones": ones,
            }
        )
    return in_maps


_CACHE: dict = {}


def _get_nc(cc: bool = True):
    key = ("nc", cc)
    if key not in _CACHE:
        _CACHE[key] = build_nc(cc=cc)
    return _CACHE[key]


def run(x, w_attn, w_proj, cc: bool = True, **spmd_kwargs):
    nc = _get_nc(cc=cc)
    in_maps = prep_inputs(
        np.asarray(x, dtype=np.float32),
        np.asarray(w_attn, dtype=np.float32),
        np.asarray(w_proj, dtype=np.float32),
    )
    res = run_bass_kernel_spmd(nc, in_maps, list(range(NCORES)), **spmd_kwargs)
    # rs_out[c][ch] holds tokens [64c .. 64c+64) of chunk ch (for the last
    # chunk: tokens [16c .. 16c+16) of each 128-token tile)
    out = np.zeros((BT, C), dtype=np.float32)
    for c in range(NCORES):
        r = np.asarray(res.results[c]["rs_out"], dtype=np.float32)
        for ch in range(NCH - 1):
            t0 = ch * TCH + c * TPC
            out[t0 : t0 + TPC, :] = r[ch].reshape(TPC, C)
        ch = NCH - 1
        rl = r[ch].reshape(4, 128 // NCORES, C)
        for tt in range(4):
            t0 = ch * TCH + tt * 128 + c * (128 // NCORES)
            out[t0 : t0 + 128 // NCORES, :] = rl[tt]
    return out.reshape(B, T, C), res


def kernel(x, w_attn, w_proj):
    out, _ = run(x, w_attn, w_proj, cc=True)
    return out
